# revision 3
# baseline (speedup 1.0000x reference)
"""Multi-head attention (B=2, S=2048, D=1024, H=16, d_k=64) on 8 Trainium2
NeuronCores — all-bf16, software-pipelined v4.

Sharding: core = b * 4 + g  (b = batch, g = head-group of 4 heads).
Each core projects Q/K/V for its 4 heads (column-sharded Wq/Wk/Wv), runs
masked softmax attention, and computes a partial output projection with the
row-shard of Wo.  The host sums the 4 partials per batch and adds bo.

v4 layout of work (the scalar engine's 72 exp tiles are the critical
resource, so everything is arranged to start them early and stream them
back-to-back):
  1. K^T projection first (streams xk), then Q^T projection (streams xq).
  2. Attention head-steps are software-pipelined one step deep: scores+exp
     for step i run while the PV matmuls of step i-1 accumulate.
  3. The V projection is interleaved into step 0's score loop, filling the
     PE gaps while the first exps run.  V-proj PSUM tiles share the score
     pool's slots (same tag) so everything fits in 8 PSUM banks.
  4. The V bias is folded into the projection as a kones-row rank-1 matmul
     (bias only lands on unmasked keys, so no separate mask multiply).
  5. The output projection for a q half is emitted right after its last
     head's normalize and drains the exp backlog of the next head-step.

Mask handling: the key mask is applied on the host by gathering only the
unmasked key columns.  Keys are padded to a multiple of 128; pad keys have
zero x-columns and a zero kones entry, so V rows, the denominator column,
and the numerator all get exact zero contributions from padding.
"""

import sys
import types

sys.path.insert(0, "/opt/trn_rl_repo")

if "antenv.axon_hooks" not in sys.modules:
    try:
        import antenv.axon_hooks  # noqa: F401
    except ImportError:
        _hooks_mod = types.ModuleType("antenv.axon_hooks")
        _hooks_mod._hook = None

        def _set_hook(h, _m=_hooks_mod):
            _m._hook = h

        def _get_hook(_m=_hooks_mod):
            return _m._hook

        _hooks_mod.set_axon_ntff_profile_hook = _set_hook
        _hooks_mod.get_axon_ntff_profile_hook = _get_hook
        sys.modules["antenv.axon_hooks"] = _hooks_mod
        try:
            import antenv as _antenv

            _antenv.axon_hooks = _hooks_mod
        except ImportError:
            pass

import ml_dtypes
import numpy as np

import concourse.bass as bass  # noqa: F401
import concourse.mybir as mybir
import concourse.tile as tile
from concourse import bacc

F32 = mybir.dt.float32
BF16 = mybir.dt.bfloat16
AF = mybir.ActivationFunctionType
ALU = mybir.AluOpType
NPBF16 = ml_dtypes.bfloat16

D = 1024
S = 2048
HL = 4
DK = 64
J = HL * DK
DC = D // 128
JC = J // 128
B = 2
GROUPS = 4
NCORES = B * GROUPS

# per-head offsets inside a V stationary tile [128, 386]:
#   even heads: [V(64) | kones]              -> psum rows 0..63 data, 64 denom
#   odd heads:  [zeros(63) | kones | V(64)]  -> psum rows 64..127 data, 32 denom
V_REGION = [(0, 65), (65, 128), (193, 65), (258, 128)]
V_DATA = [0, 129, 193, 322]
V_ONES = [64, 97, 257, 290]
V_WIDTH = 386


def build_program(kt_tiles: int):
    k_pad = kt_tiles * 128
    nc = bacc.Bacc()

    xq = nc.declare_dram_parameter("xq", [D, S], BF16, isOutput=False)
    xk = nc.declare_dram_parameter("xk", [D, k_pad], BF16, isOutput=False)
    xv = nc.declare_dram_parameter("xv", [128, DC * k_pad], BF16, isOutput=False)
    wq = nc.declare_dram_parameter("wq", [128, DC * J], BF16, isOutput=False)
    wk = nc.declare_dram_parameter("wk", [128, DC * J], BF16, isOutput=False)
    wv = nc.declare_dram_parameter("wv", [128, DC * J], BF16, isOutput=False)
    wo = nc.declare_dram_parameter("wo", [128, JC * D], BF16, isOutput=False)
    bq = nc.declare_dram_parameter("bq", [J], F32, isOutput=False)
    bk = nc.declare_dram_parameter("bk", [J], F32, isOutput=False)
    bvr = nc.declare_dram_parameter("bvr", [J], BF16, isOutput=False)
    kdiag = nc.declare_dram_parameter("kdiag", [128, k_pad], BF16, isOutput=False)
    kones = nc.declare_dram_parameter("kones", [k_pad], F32, isOutput=False)
    y = nc.declare_dram_parameter("y", [S, D], BF16, isOutput=True)

    with tile.TileContext(nc) as tc:
        with (
            tc.tile_pool(name="const", bufs=1) as cpool,
            tc.tile_pool(name="big", bufs=1) as big,
            tc.tile_pool(name="xin", bufs=3) as xin,
            tc.tile_pool(name="ptile", bufs=12) as ppool,
            tc.tile_pool(name="small", bufs=3) as small,
        ):
            qt_sb = [big.tile([128, S], BF16, tag=f"qt{jc}", name=f"qt{jc}") for jc in range(JC)]
            kt_sb = [big.tile([128, k_pad], BF16, tag=f"kt{h}", name=f"kt{h}") for h in range(HL)]
            for h in range(HL):
                po0 = 64 if h % 2 == 0 else 0
                nc.vector.memset(kt_sb[h][po0 : po0 + 64, :], 0.0)
            at_sb = [big.tile([128, S], BF16, tag=f"at{jc}", name=f"at{jc}") for jc in range(JC)]
            v_sb = [big.tile([128, V_WIDTH], BF16, tag=f"v{kt}", name=f"v{kt}") for kt in range(kt_tiles)]

            # ---- weights / constants (DMA queue: weights, xk, xq, xv) ------
            wk_sb = cpool.tile([128, DC, J], BF16, tag="wk", name="wk_sb")
            nc.sync.dma_start(wk_sb[:], wk.rearrange("p (c j) -> p c j", c=DC))
            bk_sb = cpool.tile([128, JC], F32, tag="bk", name="bk_sb")
            nc.sync.dma_start(bk_sb[:], bk.rearrange("(c p) -> p c", p=128))
            wq_sb = cpool.tile([128, DC, J], BF16, tag="wq", name="wq_sb")
            nc.sync.dma_start(wq_sb[:], wq.rearrange("p (c j) -> p c j", c=DC))
            bq_sb = cpool.tile([128, JC], F32, tag="bq", name="bq_sb")
            nc.sync.dma_start(bq_sb[:], bq.rearrange("(c p) -> p c", p=128))
            wv_sb = cpool.tile([128, DC, J], BF16, tag="wv", name="wv_sb")
            nc.sync.dma_start(wv_sb[:], wv.rearrange("p (c j) -> p c j", c=DC))
            wo_sb = cpool.tile([128, JC, D], BF16, tag="wo", name="wo_sb")
            nc.sync.dma_start(wo_sb[:], wo.rearrange("p (c m) -> p c m", c=JC))
            bvb_bc = cpool.tile([128, J], BF16, tag="bv", name="bvb_bc")
            nc.sync.dma_start(bvb_bc[:], bvr.ap()[None, :].to_broadcast((128, J)))
            kdiag_sb = cpool.tile([128, k_pad], BF16, tag="kdiag", name="kdiag_sb")
            nc.sync.dma_start(kdiag_sb[:], kdiag[:, :])
            kones_sb = cpool.tile([128, kt_tiles], F32, tag="kones", name="kones_sb")
            nc.sync.dma_start(kones_sb[:], kones.rearrange("(t p) -> p t", p=128))
            z64 = cpool.tile([128, 64], BF16, tag="z64", name="z64")
            nc.vector.memset(z64[:], 0.0)

            kchunks = []
            off = 0
            while off < k_pad:
                w = min(512, k_pad - off)
                kchunks.append((off, w))
                off += w

            with tc.tile_pool(name="proj_psum", bufs=1, space="PSUM") as pp:
                # ---- K^T projection (streams xk) ---------------------------
                psk = [
                    pp.tile([128, 512], F32, tag=f"psq{i}", name=f"psk{i}")
                    for i in range(JC * len(kchunks))
                ]
                for dc in range(DC):
                    xk_t = xin.tile([128, k_pad], BF16, tag="xk", name="xk_t")
                    nc.sync.dma_start(xk_t[:], xk[dc * 128 : (dc + 1) * 128, :])
                    for jc in range(JC):
                        lhsT = wk_sb[:, dc, jc * 128 : (jc + 1) * 128]
                        for i, (off, w) in enumerate(kchunks):
                            nc.tensor.matmul(
                                psk[jc * len(kchunks) + i][:, :w],
                                lhsT,
                                xk_t[:, off : off + w],
                                start=(dc == 0),
                                stop=(dc == DC - 1),
                            )
                for jc in range(JC):
                    for i, (off, w) in enumerate(kchunks):
                        ps_ = psk[jc * len(kchunks) + i]
                        nc.vector.tensor_tensor(
                            kt_sb[2 * jc][0:64, off : off + w],
                            ps_[0:64, :w],
                            bk_sb[0:64, jc : jc + 1].to_broadcast((64, w)),
                            ALU.add,
                        )
                        nc.vector.tensor_tensor(
                            kt_sb[2 * jc + 1][64:128, off : off + w],
                            ps_[64:128, :w],
                            bk_sb[64:128, jc : jc + 1].to_broadcast((64, w)),
                            ALU.add,
                        )

                # ---- Q^T projection (streams xq) ---------------------------
                QQC = S // 512
                psq = [pp.tile([128, 512], F32, tag=f"psq{i}", name=f"psq{i}") for i in range(JC * QQC)]
                for dc in range(DC):
                    xq_t = xin.tile([128, S], BF16, tag="xq", name="xq_t")
                    nc.sync.dma_start(xq_t[:], xq[dc * 128 : (dc + 1) * 128, :])
                    for jc in range(JC):
                        lhsT = wq_sb[:, dc, jc * 128 : (jc + 1) * 128]
                        for qc in range(QQC):
                            nc.tensor.matmul(
                                psq[jc * QQC + qc][:],
                                lhsT,
                                xq_t[:, qc * 512 : (qc + 1) * 512],
                                start=(dc == 0),
                                stop=(dc == DC - 1),
                            )
                for jc in range(JC):
                    for qc in range(QQC):
                        nc.vector.tensor_tensor(
                            qt_sb[jc][:, qc * 512 : (qc + 1) * 512],
                            psq[jc * QQC + qc][:],
                            bq_sb[:, jc : jc + 1].to_broadcast((128, 512)),
                            ALU.add,
                        )

            # xv lands after xq on the DMA queue; its projection is emitted
            # inside attention step 0 below.
            xv_sb = big.tile([128, DC, k_pad], BF16, tag="xv", name="xv_sb")
            nc.sync.dma_start(xv_sb[:], xv.rearrange("p (c k) -> p c k", c=DC))

            # ---- attention, software-pipelined one head-step deep ----------
            QH = S // 1024
            seq = [(qh, h) for qh in range(QH) for h in range(HL)]
            pt_tiles: dict = {}
            sp = ap = None  # bound by the with-block below

            def emit_score_kt(idx, kt):
                qh, h = seq[idx]
                jc = h // 2
                ps = sp.tile([128, 1024], F32, tag="ps", name="ps")
                lhs_k = kt_sb[h][:, kt * 128 : (kt + 1) * 128]
                for i in range(2):
                    nc.tensor.matmul(
                        ps[:, i * 512 : (i + 1) * 512],
                        lhs_k,
                        qt_sb[jc][:, qh * 1024 + i * 512 : qh * 1024 + (i + 1) * 512],
                        start=True,
                        stop=True,
                    )
                pt = ppool.tile([128, 1024], BF16, tag="pt", name="pt")
                nc.scalar.activation(pt[:], ps[:], AF.Exp, scale=0.125)
                pt_tiles[(idx, kt)] = pt

            def emit_vproj_kt(kt):
                # V projection for one key tile; PSUM slot shared with the
                # score tiles ("ps" tag).  Bias lands via a kones-row rank-1
                # matmul, so pad keys stay exactly zero and no separate mask
                # multiply is needed.
                psv_t = sp.tile([128, 1024], F32, tag="ps", name="psv")
                psv = psv_t[:, 0:J]
                for dc in range(DC):
                    nc.tensor.matmul(
                        psv,
                        xv_sb[:, dc, kt * 128 : (kt + 1) * 128],
                        wv_sb[:, dc, :],
                        start=(dc == 0),
                        stop=False,
                    )
                # bias via a diagonal-kones stationary: psv[m,:] += kones[m]*bv
                # (full 128x128 stationary keeps the PE tile config uniform
                # across the accumulation group; pad keys get no bias)
                nc.tensor.matmul(
                    psv,
                    kdiag_sb[:, kt * 128 : (kt + 1) * 128],
                    bvb_bc[:, :],
                    start=False,
                    stop=True,
                )
                vt = v_sb[kt]
                nc.vector.tensor_copy(vt[:, 65:129], z64[:])
                nc.vector.tensor_copy(vt[:, 258:322], z64[:])
                kcol = kones_sb[:, kt : kt + 1]
                for h in range(HL):
                    d0 = V_DATA[h]
                    nc.vector.tensor_copy(vt[:, d0 : d0 + DK], psv_t[:, h * DK : (h + 1) * DK])
                    nc.vector.tensor_copy(vt[:, V_ONES[h] : V_ONES[h] + 1], kcol)

            def emit_pv(idx):
                qh, h = seq[idx]
                vstart, vwidth = V_REGION[h]
                aug = ap.tile([128, 1024], F32, tag="aug", name="aug")
                for kt in range(kt_tiles):
                    pt = pt_tiles.pop((idx, kt))
                    lhs_v = v_sb[kt][:, vstart : vstart + vwidth]
                    for i in range(2):
                        nc.tensor.matmul(
                            aug[:vwidth, i * 512 : (i + 1) * 512],
                            lhs_v,
                            pt[:, i * 512 : (i + 1) * 512],
                            start=(kt == 0),
                            stop=(kt == kt_tiles - 1),
                        )
                return aug

            def emit_norm(idx, aug):
                qh, h = seq[idx]
                jc = h // 2
                po = (h % 2) * 64
                den = 64 if h % 2 == 0 else 32
                # custom-DVE ops read garbage from PSUM: stage the denominator
                # row through SBUF with a plain copy first
                dt_ = small.tile([1, 1024], F32, tag="dt", name="dt")
                nc.vector.tensor_copy(dt_[:], aug[den : den + 1, :])
                rt = small.tile([1, 1024], F32, tag="rt", name="rt")
                nc.vector.reciprocal_approx_fast(rt[:], dt_[:])
                rb = small.tile([128, 1024], F32, tag="rb", name="rb")
                nc.gpsimd.partition_broadcast(rb[:], rt[:])
                nc.vector.tensor_tensor(
                    at_sb[jc][po : po + DK, qh * 1024 : (qh + 1) * 1024],
                    aug[po : po + DK, :],
                    rb[po : po + DK, :],
                    ALU.mult,
                )

            def emit_oproj(qh):
                for qt in range(qh * 8, (qh + 1) * 8):
                    psy = ap.tile([128, 1024], F32, tag="aug", name="psy")
                    for jc in range(JC):
                        lhsT = at_sb[jc][:, qt * 128 : (qt + 1) * 128]
                        for mc in range(2):
                            nc.tensor.matmul(
                                psy[:, mc * 512 : (mc + 1) * 512],
                                lhsT,
                                wo_sb[:, jc, mc * 512 : (mc + 1) * 512],
                                start=(jc == 0),
                                stop=(jc == JC - 1),
                            )
                    yt = small.tile([128, 1024], BF16, tag="yt", name="yt")
                    nc.vector.tensor_copy(yt[:], psy[:])
                    nc.sync.dma_start(y[qt * 128 : (qt + 1) * 128, :], yt[:])

            with (
                tc.tile_pool(name="score_psum", bufs=2, space="PSUM") as sp,
                tc.tile_pool(name="aug_psum", bufs=2, space="PSUM") as ap,
            ):
                for kt in range(kt_tiles):
                    emit_score_kt(0, kt)
                    emit_vproj_kt(kt)
                for idx in range(1, len(seq)):
                    for kt in range(kt_tiles):
                        emit_score_kt(idx, kt)
                    aug_prev = emit_pv(idx - 1)
                    emit_norm(idx - 1, aug_prev)
                    pqh, ph = seq[idx - 1]
                    if ph == HL - 1:
                        emit_oproj(pqh)
                aug_last = emit_pv(len(seq) - 1)
                emit_norm(len(seq) - 1, aug_last)
                emit_oproj(QH - 1)

    nc.finalize()
    return nc


_CACHE: dict = {}


def _get_program(kt_tiles: int):
    if kt_tiles not in _CACHE:
        _CACHE[kt_tiles] = build_program(kt_tiles)
    return _CACHE[kt_tiles]


def _pack_w(wT):
    """[D, N] (contraction-major) -> [128, (D//128)*N] partition-packed."""
    Dd, N = wT.shape
    return np.ascontiguousarray(
        wT.reshape(Dd // 128, 128, N).transpose(1, 0, 2).reshape(128, -1)
    )


def _prep_inputs(q, k, v, mask, Wq, bq, Wk, bk, Wv, bv, Wo, bo):
    """Shard + transpose + compact on the host. Returns (in_maps, kt_tiles)."""
    idx = [np.nonzero(mask[b])[0] for b in range(B)]
    s_u = max(1, max(len(i) for i in idx))
    kt_tiles = (s_u + 127) // 128
    k_pad = kt_tiles * 128

    per_batch = []
    for b in range(B):
        qT = np.ascontiguousarray(q[b].T.astype(NPBF16))
        kT = np.zeros((D, k_pad), NPBF16)
        vT = np.zeros((D, k_pad), NPBF16)
        n = len(idx[b])
        kT[:, :n] = k[b].T[:, idx[b]].astype(NPBF16)
        vT[:, :n] = v[b].T[:, idx[b]].astype(NPBF16)
        ko = np.zeros((k_pad,), np.float32)
        ko[:n] = 1.0
        kd = np.zeros((128, k_pad), NPBF16)
        for t in range(kt_tiles):
            np.fill_diagonal(kd[:, t * 128 : (t + 1) * 128], ko[t * 128 : (t + 1) * 128])
        vTp = _pack_w(vT)
        per_batch.append((qT, kT, vTp, ko, kd))

    in_maps = []
    for core in range(NCORES):
        b, g = divmod(core, GROUPS)
        j0 = g * J
        qT, kT, vTp, ko, kd = per_batch[b]
        in_maps.append(
            {
                "xq": qT,
                "xk": kT,
                "xv": vTp,
                "wq": _pack_w(Wq[j0 : j0 + J, :].T.astype(NPBF16)),
                "wk": _pack_w(Wk[j0 : j0 + J, :].T.astype(NPBF16)),
                "wv": _pack_w(Wv[j0 : j0 + J, :].T.astype(NPBF16)),
                "wo": _pack_w(Wo[:, j0 : j0 + J].T.astype(NPBF16)),
                "bq": np.ascontiguousarray(bq[j0 : j0 + J]).astype(np.float32),
                "bk": np.ascontiguousarray(bk[j0 : j0 + J]).astype(np.float32),
                "bvr": np.ascontiguousarray(bv[j0 : j0 + J]).astype(NPBF16),
                "kones": ko,
                "kdiag": kd,
            }
        )
    return in_maps, kt_tiles


def run(inputs: dict, trace: bool = False):
    """Run the sharded kernel; returns (output [B,S,D] f32, BassKernelResults)."""
    from concourse.bass_utils import run_bass_kernel_spmd

    inputs = {k: np.asarray(v) for k, v in inputs.items()}
    in_maps, kt_tiles = _prep_inputs(**inputs)
    nc = _get_program(kt_tiles)
    res = run_bass_kernel_spmd(nc, in_maps, list(range(NCORES)), trace=trace)
    bo = inputs["bo"].astype(np.float32)
    out = np.empty((B, S, D), np.float32)
    for b in range(B):
        acc = np.zeros((S, D), np.float32)
        for g in range(GROUPS):
            acc += np.asarray(res.results[b * GROUPS + g]["y"], np.float32)
        out[b] = acc + bo[None, :]
    return out, res


def kernel(**inputs) -> np.ndarray:
    out, _ = run(inputs, trace=False)
    return out


# revision 4
# speedup vs baseline: 1.0064x; 1.0064x over previous
"""Multi-head attention (B=2, S=2048, D=1024, H=16, d_k=64) on 8 Trainium2
NeuronCores — all-bf16, software-pipelined v4.

Sharding: core = b * 4 + g  (b = batch, g = head-group of 4 heads).
Each core projects Q/K/V for its 4 heads (column-sharded Wq/Wk/Wv), runs
masked softmax attention, and computes a partial output projection with the
row-shard of Wo.  The host sums the 4 partials per batch and adds bo.

v4 layout of work (the scalar engine's 72 exp tiles are the critical
resource, so everything is arranged to start them early and stream them
back-to-back):
  1. K^T projection first (streams xk), then Q^T projection (streams xq).
  2. Attention head-steps are software-pipelined one step deep: scores+exp
     for step i run while the PV matmuls of step i-1 accumulate.
  3. The V projection is interleaved into step 0's score loop, filling the
     PE gaps while the first exps run.  V-proj PSUM tiles share the score
     pool's slots (same tag) so everything fits in 8 PSUM banks.
  4. The V bias is folded into the projection as a kones-row rank-1 matmul
     (bias only lands on unmasked keys, so no separate mask multiply).
  5. The output projection for a q half is emitted right after its last
     head's normalize and drains the exp backlog of the next head-step.

Mask handling: the key mask is applied on the host by gathering only the
unmasked key columns.  Keys are padded to a multiple of 128; pad keys have
zero x-columns and a zero kones entry, so V rows, the denominator column,
and the numerator all get exact zero contributions from padding.
"""

import sys
import types

sys.path.insert(0, "/opt/trn_rl_repo")

if "antenv.axon_hooks" not in sys.modules:
    try:
        import antenv.axon_hooks  # noqa: F401
    except ImportError:
        _hooks_mod = types.ModuleType("antenv.axon_hooks")
        _hooks_mod._hook = None

        def _set_hook(h, _m=_hooks_mod):
            _m._hook = h

        def _get_hook(_m=_hooks_mod):
            return _m._hook

        _hooks_mod.set_axon_ntff_profile_hook = _set_hook
        _hooks_mod.get_axon_ntff_profile_hook = _get_hook
        sys.modules["antenv.axon_hooks"] = _hooks_mod
        try:
            import antenv as _antenv

            _antenv.axon_hooks = _hooks_mod
        except ImportError:
            pass

import ml_dtypes
import numpy as np

import concourse.bass as bass  # noqa: F401
import concourse.mybir as mybir
import concourse.tile as tile
from concourse import bacc

F32 = mybir.dt.float32
BF16 = mybir.dt.bfloat16
AF = mybir.ActivationFunctionType
ALU = mybir.AluOpType
NPBF16 = ml_dtypes.bfloat16

D = 1024
S = 2048
HL = 4
DK = 64
J = HL * DK
DC = D // 128
JC = J // 128
B = 2
GROUPS = 4
NCORES = B * GROUPS

# per-head offsets inside a V stationary tile [128, 386]:
#   even heads: [V(64) | kones]              -> psum rows 0..63 data, 64 denom
#   odd heads:  [zeros(63) | kones | V(64)]  -> psum rows 64..127 data, 32 denom
V_REGION = [(0, 65), (65, 128), (193, 65), (258, 128)]
V_DATA = [0, 129, 193, 322]
V_ONES = [64, 97, 257, 290]
V_WIDTH = 386


def build_program(kt_tiles: int):
    k_pad = kt_tiles * 128
    nc = bacc.Bacc()

    xqlo = nc.declare_dram_parameter("xqlo", [128, DC * 1024], BF16, isOutput=False)
    xqhi = nc.declare_dram_parameter("xqhi", [128, DC * 1024], BF16, isOutput=False)
    xk = nc.declare_dram_parameter("xk", [D, k_pad], BF16, isOutput=False)
    xv = nc.declare_dram_parameter("xv", [128, DC * k_pad], BF16, isOutput=False)
    wq = nc.declare_dram_parameter("wq", [128, DC * J], BF16, isOutput=False)
    wk = nc.declare_dram_parameter("wk", [128, DC * J], BF16, isOutput=False)
    wv = nc.declare_dram_parameter("wv", [128, DC * J], BF16, isOutput=False)
    wo = nc.declare_dram_parameter("wo", [128, JC * D], BF16, isOutput=False)
    bq = nc.declare_dram_parameter("bq", [J], F32, isOutput=False)
    bk = nc.declare_dram_parameter("bk", [J], F32, isOutput=False)
    bvr = nc.declare_dram_parameter("bvr", [J], BF16, isOutput=False)
    kdiag = nc.declare_dram_parameter("kdiag", [128, k_pad], BF16, isOutput=False)
    kones = nc.declare_dram_parameter("kones", [k_pad], F32, isOutput=False)
    y = nc.declare_dram_parameter("y", [S, D], BF16, isOutput=True)

    with tile.TileContext(nc) as tc:
        with (
            tc.tile_pool(name="const", bufs=1) as cpool,
            tc.tile_pool(name="big", bufs=1) as big,
            tc.tile_pool(name="xin", bufs=3) as xin,
            tc.tile_pool(name="ptile", bufs=12) as ppool,
            tc.tile_pool(name="small", bufs=3) as small,
        ):
            qt_sb = [big.tile([128, S], BF16, tag=f"qt{jc}", name=f"qt{jc}") for jc in range(JC)]
            kt_sb = [big.tile([128, k_pad], BF16, tag=f"kt{h}", name=f"kt{h}") for h in range(HL)]
            for h in range(HL):
                po0 = 64 if h % 2 == 0 else 0
                nc.vector.memset(kt_sb[h][po0 : po0 + 64, :], 0.0)
            at_sb = [big.tile([128, S], BF16, tag=f"at{jc}", name=f"at{jc}") for jc in range(JC)]
            v_sb = [big.tile([128, V_WIDTH], BF16, tag=f"v{kt}", name=f"v{kt}") for kt in range(kt_tiles)]

            # ---- weights / constants (DMA queue: weights, xk, xq, xv) ------
            wk_sb = cpool.tile([128, DC, J], BF16, tag="wk", name="wk_sb")
            nc.sync.dma_start(wk_sb[:], wk.rearrange("p (c j) -> p c j", c=DC))
            bk_sb = cpool.tile([128, JC], F32, tag="bk", name="bk_sb")
            nc.sync.dma_start(bk_sb[:], bk.rearrange("(c p) -> p c", p=128))
            wq_sb = cpool.tile([128, DC, J], BF16, tag="wq", name="wq_sb")
            nc.sync.dma_start(wq_sb[:], wq.rearrange("p (c j) -> p c j", c=DC))
            bq_sb = cpool.tile([128, JC], F32, tag="bq", name="bq_sb")
            nc.sync.dma_start(bq_sb[:], bq.rearrange("(c p) -> p c", p=128))
            z64 = cpool.tile([128, 64], BF16, tag="z64", name="z64")
            nc.vector.memset(z64[:], 0.0)

            kchunks = []
            off = 0
            while off < k_pad:
                w = min(512, k_pad - off)
                kchunks.append((off, w))
                off += w

            with tc.tile_pool(name="proj_psum", bufs=1, space="PSUM") as pp:
                # ---- K^T projection (streams xk) ---------------------------
                psk = [
                    pp.tile([128, 512], F32, tag=f"psq{i}", name=f"psk{i}")
                    for i in range(JC * len(kchunks))
                ]
                for dc in range(DC):
                    xk_t = xin.tile([128, k_pad], BF16, tag="xk", name="xk_t")
                    nc.sync.dma_start(xk_t[:], xk[dc * 128 : (dc + 1) * 128, :])
                    for jc in range(JC):
                        lhsT = wk_sb[:, dc, jc * 128 : (jc + 1) * 128]
                        for i, (off, w) in enumerate(kchunks):
                            nc.tensor.matmul(
                                psk[jc * len(kchunks) + i][:, :w],
                                lhsT,
                                xk_t[:, off : off + w],
                                start=(dc == 0),
                                stop=(dc == DC - 1),
                            )
                for jc in range(JC):
                    for i, (off, w) in enumerate(kchunks):
                        ps_ = psk[jc * len(kchunks) + i]
                        nc.vector.tensor_tensor(
                            kt_sb[2 * jc][0:64, off : off + w],
                            ps_[0:64, :w],
                            bk_sb[0:64, jc : jc + 1].to_broadcast((64, w)),
                            ALU.add,
                        )
                        nc.vector.tensor_tensor(
                            kt_sb[2 * jc + 1][64:128, off : off + w],
                            ps_[64:128, :w],
                            bk_sb[64:128, jc : jc + 1].to_broadcast((64, w)),
                            ALU.add,
                        )

                # ---- Q^T projection, first q half only (resident xqlo) -----
                xqlo_sb = big.tile([128, DC, 1024], BF16, tag="xqlo", name="xqlo_sb")
                nc.sync.dma_start(xqlo_sb[:], xqlo.rearrange("p (c q) -> p c q", c=DC))
                for jc in range(JC):
                    for qc in range(2):
                        psq = pp.tile([128, 512], F32, tag=f"psq{jc * 2 + qc}", name="psq")
                        for dc in range(DC):
                            nc.tensor.matmul(
                                psq[:],
                                wq_sb[:, dc, jc * 128 : (jc + 1) * 128],
                                xqlo_sb[:, dc, qc * 512 : (qc + 1) * 512],
                                start=(dc == 0),
                                stop=(dc == DC - 1),
                            )
                        nc.vector.tensor_tensor(
                            qt_sb[jc][:, qc * 512 : (qc + 1) * 512],
                            psq[:],
                            bq_sb[:, jc : jc + 1].to_broadcast((128, 512)),
                            ALU.add,
                        )

            # x/weight tails on the DMA queue after the critical-path loads
            xv_sb = big.tile([128, DC, k_pad], BF16, tag="xv", name="xv_sb")
            nc.sync.dma_start(xv_sb[:], xv.rearrange("p (c k) -> p c k", c=DC))
            wv_sb = cpool.tile([128, DC, J], BF16, tag="wv", name="wv_sb")
            nc.sync.dma_start(wv_sb[:], wv.rearrange("p (c j) -> p c j", c=DC))
            wo_sb = cpool.tile([128, JC, D], BF16, tag="wo", name="wo_sb")
            nc.sync.dma_start(wo_sb[:], wo.rearrange("p (c m) -> p c m", c=JC))
            bvb_bc = cpool.tile([128, J], BF16, tag="bv", name="bvb_bc")
            nc.sync.dma_start(bvb_bc[:], bvr.ap()[None, :].to_broadcast((128, J)))
            kdiag_sb = cpool.tile([128, k_pad], BF16, tag="kdiag", name="kdiag_sb")
            nc.sync.dma_start(kdiag_sb[:], kdiag[:, :])
            kones_sb = cpool.tile([128, kt_tiles], F32, tag="kones", name="kones_sb")
            nc.sync.dma_start(kones_sb[:], kones.rearrange("(t p) -> p t", p=128))
            xqhi_sb = big.tile([128, DC, 1024], BF16, tag="xqhi", name="xqhi_sb")
            nc.sync.dma_start(xqhi_sb[:], xqhi.rearrange("p (c q) -> p c q", c=DC))

            # ---- attention, software-pipelined one head-step deep ----------
            QH = S // 1024
            seq = [(qh, h) for qh in range(QH) for h in range(HL)]
            pt_tiles: dict = {}
            sp = ap = None  # bound by the with-block below

            def emit_score_kt(idx, kt):
                qh, h = seq[idx]
                jc = h // 2
                ps = sp.tile([128, 1024], F32, tag="ps", name="ps")
                lhs_k = kt_sb[h][:, kt * 128 : (kt + 1) * 128]
                for i in range(2):
                    nc.tensor.matmul(
                        ps[:, i * 512 : (i + 1) * 512],
                        lhs_k,
                        qt_sb[jc][:, qh * 1024 + i * 512 : qh * 1024 + (i + 1) * 512],
                        start=True,
                        stop=True,
                    )
                pt = ppool.tile([128, 1024], BF16, tag="pt", name="pt")
                nc.scalar.activation(pt[:], ps[:], AF.Exp, scale=0.125)
                pt_tiles[(idx, kt)] = pt

            def emit_vproj_kt(kt):
                # V projection for one key tile; PSUM slot shared with the
                # score tiles ("ps" tag).  Bias lands via a kones-row rank-1
                # matmul, so pad keys stay exactly zero and no separate mask
                # multiply is needed.
                psv_t = sp.tile([128, 1024], F32, tag="ps", name="psv")
                psv = psv_t[:, 0:J]
                for dc in range(DC):
                    nc.tensor.matmul(
                        psv,
                        xv_sb[:, dc, kt * 128 : (kt + 1) * 128],
                        wv_sb[:, dc, :],
                        start=(dc == 0),
                        stop=False,
                    )
                # bias via a diagonal-kones stationary: psv[m,:] += kones[m]*bv
                # (full 128x128 stationary keeps the PE tile config uniform
                # across the accumulation group; pad keys get no bias)
                nc.tensor.matmul(
                    psv,
                    kdiag_sb[:, kt * 128 : (kt + 1) * 128],
                    bvb_bc[:, :],
                    start=False,
                    stop=True,
                )
                vt = v_sb[kt]
                nc.vector.tensor_copy(vt[:, 65:129], z64[:])
                nc.vector.tensor_copy(vt[:, 258:322], z64[:])
                kcol = kones_sb[:, kt : kt + 1]
                for h in range(HL):
                    d0 = V_DATA[h]
                    nc.vector.tensor_copy(vt[:, d0 : d0 + DK], psv_t[:, h * DK : (h + 1) * DK])
                    nc.vector.tensor_copy(vt[:, V_ONES[h] : V_ONES[h] + 1], kcol)

            def emit_qhi_chain(jc, qc):
                # one deferred Q-projection chain for the second q half,
                # routed through the score pool's PSUM slots
                psq_t = sp.tile([128, 1024], F32, tag="ps", name="psqhi")
                psq = psq_t[:, 0:512]
                for dc in range(DC):
                    nc.tensor.matmul(
                        psq,
                        wq_sb[:, dc, jc * 128 : (jc + 1) * 128],
                        xqhi_sb[:, dc, qc * 512 : (qc + 1) * 512],
                        start=(dc == 0),
                        stop=(dc == DC - 1),
                    )
                nc.vector.tensor_tensor(
                    qt_sb[jc][:, 1024 + qc * 512 : 1024 + (qc + 1) * 512],
                    psq,
                    bq_sb[:, jc : jc + 1].to_broadcast((128, 512)),
                    ALU.add,
                )

            def emit_pv(idx):
                qh, h = seq[idx]
                vstart, vwidth = V_REGION[h]
                aug = ap.tile([128, 1024], F32, tag="aug", name="aug")
                for kt in range(kt_tiles):
                    pt = pt_tiles.pop((idx, kt))
                    lhs_v = v_sb[kt][:, vstart : vstart + vwidth]
                    for i in range(2):
                        nc.tensor.matmul(
                            aug[:vwidth, i * 512 : (i + 1) * 512],
                            lhs_v,
                            pt[:, i * 512 : (i + 1) * 512],
                            start=(kt == 0),
                            stop=(kt == kt_tiles - 1),
                        )
                return aug

            def emit_norm(idx, aug):
                qh, h = seq[idx]
                jc = h // 2
                po = (h % 2) * 64
                den = 64 if h % 2 == 0 else 32
                # custom-DVE ops read garbage from PSUM: stage the denominator
                # row through SBUF with a plain copy first
                dt_ = small.tile([1, 1024], F32, tag="dt", name="dt")
                nc.vector.tensor_copy(dt_[:], aug[den : den + 1, :])
                rt = small.tile([1, 1024], F32, tag="rt", name="rt")
                nc.vector.reciprocal_approx_fast(rt[:], dt_[:])
                rb = small.tile([128, 1024], F32, tag="rb", name="rb")
                nc.gpsimd.partition_broadcast(rb[:], rt[:])
                nc.vector.tensor_tensor(
                    at_sb[jc][po : po + DK, qh * 1024 : (qh + 1) * 1024],
                    aug[po : po + DK, :],
                    rb[po : po + DK, :],
                    ALU.mult,
                )

            def emit_oproj(qh):
                for qt in range(qh * 8, (qh + 1) * 8):
                    psy = ap.tile([128, 1024], F32, tag="aug", name="psy")
                    for jc in range(JC):
                        lhsT = at_sb[jc][:, qt * 128 : (qt + 1) * 128]
                        for mc in range(2):
                            nc.tensor.matmul(
                                psy[:, mc * 512 : (mc + 1) * 512],
                                lhsT,
                                wo_sb[:, jc, mc * 512 : (mc + 1) * 512],
                                start=(jc == 0),
                                stop=(jc == JC - 1),
                            )
                    yt = small.tile([128, 1024], BF16, tag="yt", name="yt")
                    nc.vector.tensor_copy(yt[:], psy[:])
                    nc.sync.dma_start(y[qt * 128 : (qt + 1) * 128, :], yt[:])

            with (
                tc.tile_pool(name="score_psum", bufs=2, space="PSUM") as sp,
                tc.tile_pool(name="aug_psum", bufs=2, space="PSUM") as ap,
            ):
                for kt in range(kt_tiles):
                    emit_score_kt(0, kt)
                    emit_vproj_kt(kt)
                qhi_chains = [(jc, qc) for jc in range(JC) for qc in range(2)]
                for idx in range(1, len(seq)):
                    for kt in range(kt_tiles):
                        emit_score_kt(idx, kt)
                        if idx in (1, 2) and kt in (2, 6) and qhi_chains:
                            emit_qhi_chain(*qhi_chains.pop(0))
                    aug_prev = emit_pv(idx - 1)
                    emit_norm(idx - 1, aug_prev)
                    pqh, ph = seq[idx - 1]
                    if ph == HL - 1:
                        emit_oproj(pqh)
                aug_last = emit_pv(len(seq) - 1)
                emit_norm(len(seq) - 1, aug_last)
                emit_oproj(QH - 1)

    nc.finalize()
    return nc


_CACHE: dict = {}


def _get_program(kt_tiles: int):
    if kt_tiles not in _CACHE:
        _CACHE[kt_tiles] = build_program(kt_tiles)
    return _CACHE[kt_tiles]


def _pack_w(wT):
    """[D, N] (contraction-major) -> [128, (D//128)*N] partition-packed."""
    Dd, N = wT.shape
    return np.ascontiguousarray(
        wT.reshape(Dd // 128, 128, N).transpose(1, 0, 2).reshape(128, -1)
    )


def _prep_inputs(q, k, v, mask, Wq, bq, Wk, bk, Wv, bv, Wo, bo):
    """Shard + transpose + compact on the host. Returns (in_maps, kt_tiles)."""
    idx = [np.nonzero(mask[b])[0] for b in range(B)]
    s_u = max(1, max(len(i) for i in idx))
    kt_tiles = (s_u + 127) // 128
    k_pad = kt_tiles * 128

    per_batch = []
    for b in range(B):
        qT = q[b].T.astype(NPBF16)
        qlo = _pack_w(np.ascontiguousarray(qT[:, 0:1024]))
        qhi = _pack_w(np.ascontiguousarray(qT[:, 1024:2048]))
        kT = np.zeros((D, k_pad), NPBF16)
        vT = np.zeros((D, k_pad), NPBF16)
        n = len(idx[b])
        kT[:, :n] = k[b].T[:, idx[b]].astype(NPBF16)
        vT[:, :n] = v[b].T[:, idx[b]].astype(NPBF16)
        ko = np.zeros((k_pad,), np.float32)
        ko[:n] = 1.0
        kd = np.zeros((128, k_pad), NPBF16)
        for t in range(kt_tiles):
            np.fill_diagonal(kd[:, t * 128 : (t + 1) * 128], ko[t * 128 : (t + 1) * 128])
        vTp = _pack_w(vT)
        per_batch.append((qlo, qhi, kT, vTp, ko, kd))

    in_maps = []
    for core in range(NCORES):
        b, g = divmod(core, GROUPS)
        j0 = g * J
        qlo, qhi, kT, vTp, ko, kd = per_batch[b]
        in_maps.append(
            {
                "xqlo": qlo,
                "xqhi": qhi,
                "xk": kT,
                "xv": vTp,
                "wq": _pack_w(Wq[j0 : j0 + J, :].T.astype(NPBF16)),
                "wk": _pack_w(Wk[j0 : j0 + J, :].T.astype(NPBF16)),
                "wv": _pack_w(Wv[j0 : j0 + J, :].T.astype(NPBF16)),
                "wo": _pack_w(Wo[:, j0 : j0 + J].T.astype(NPBF16)),
                "bq": np.ascontiguousarray(bq[j0 : j0 + J]).astype(np.float32),
                "bk": np.ascontiguousarray(bk[j0 : j0 + J]).astype(np.float32),
                "bvr": np.ascontiguousarray(bv[j0 : j0 + J]).astype(NPBF16),
                "kones": ko,
                "kdiag": kd,
            }
        )
    return in_maps, kt_tiles


def run(inputs: dict, trace: bool = False):
    """Run the sharded kernel; returns (output [B,S,D] f32, BassKernelResults)."""
    from concourse.bass_utils import run_bass_kernel_spmd

    inputs = {k: np.asarray(v) for k, v in inputs.items()}
    in_maps, kt_tiles = _prep_inputs(**inputs)
    nc = _get_program(kt_tiles)
    res = run_bass_kernel_spmd(nc, in_maps, list(range(NCORES)), trace=trace)
    bo = inputs["bo"].astype(np.float32)
    out = np.empty((B, S, D), np.float32)
    for b in range(B):
        acc = np.zeros((S, D), np.float32)
        for g in range(GROUPS):
            acc += np.asarray(res.results[b * GROUPS + g]["y"], np.float32)
        out[b] = acc + bo[None, :]
    return out, res


def kernel(**inputs) -> np.ndarray:
    out, _ = run(inputs, trace=False)
    return out
